# revision 51
# baseline (speedup 1.0000x reference)
"""GCN (2-layer, Citeseer-style) on 8 Trainium2 NeuronCores.

Strategy (dst-node graph partitioning, per the sharding hint):
  - 50000 nodes -> 392 bins of <=128 nodes (degree-balanced), 49 bins/core.
  - Phase 1: support = x_shard @ W1 per core. x is host-transposed,
    stored fp8-e4m3, streamed in 512-node strips; W1 is fp8 scaled x64;
    fp8 DoubleRow matmuls (K=256/instruction) accumulate in fp32 PSUM.
    Support is stored bf16 (still x64-scaled; the scale is undone for
    free in the L1 leaky_relu's scale parameter).
  - The support table is AllGathered in two row-halves (A: first 3584
    rows/core, B: last 2688) so AG_A + early table-A gathers overlap
    phase 1; the asymmetric A/B split keeps gather indices within int16
    range while minimizing CL+CU (=9 chunks per tile).
  - Phase 3 (L1 aggregation): batched dma_gathers (8 chunks x 128 edges
    = 1024 indices per call, the hardware max) fetch bf16 source rows
    (512B) from the A/B tables; a selection matrix
    Mt[e,d] = w_e * (dst_e == d) is built bf16 by one fused
    tensor_scalar against a constant iota; two matmuls per chunk
    (lhsT=G halves, rhs=Mt) accumulate the TRANSPOSED aggregate
    zT[f,d] in two separate PSUM banks (interleaved accumulation groups
    must not share a 2KB zero-region); bias + leaky_relu -> h1T (bf16,
    SBUF-resident).
  - Phase 5 (fused into the phase-3 tile loop): s2 tile = h1 @ W2 via
    matmul(lhsT=h1T slice, rhs=W2 half) - no transposes needed. s2 rows
    land in a p-major bf16 table (256B stride, 16B valid), DMA'd +
    AllGathered in A/B halves at tiles 27/48 so the A-half L2 gathers
    overlap phase 3's tail.
  - Phase 6 (L2 aggregation): same Mt machinery, lhsT=Mt, rhs=gathered
    s2 rows (N=6); + b2; log_softmax + output DMA run in 3 groups so
    the tail overlaps the last gathers.
"""
import sys

sys.path.insert(0, "/opt/trn_rl_repo")

import numpy as np

from concourse import bass, bacc, mybir, tile
from concourse.bass_utils import run_bass_kernel_spmd

F32 = mybir.dt.float32
BF16 = mybir.dt.bfloat16
FP8 = mybir.dt.float8e4
I16 = mybir.dt.int16
X_FP8 = True               # store x in fp8 e4m3 (phase-1 lhsT)
DR = True                  # fp8 DoubleRow phase-1 matmuls (W1 fp8 scaled x64)
W1_SCALE = 64.0            # pre-scale on W1 so fp8 e4m3 holds it; support
                           # table is stored scaled, undone in the L1 lrelu
LRELU = True               # False -> plain Relu (CoreSim lacks Lrelu)

N_NODES = 50000
N_EDGES = 400000
F_IN = 3703
F_HID = 256
F_OUT = 6

CORES = 8
P = 128
TILES = 49                 # dst tiles per core
NPC = TILES * P            # 6272 padded nodes per core
NTOT = CORES * NPC         # 50176
TILES_A = 28               # tiles in the A (first) half
NPCA = TILES_A * P         # 3584 first-half rows per core (A table)
TILES_B = TILES - TILES_A  # 21
NPCB = NPC - NPCA          # 2688 (B table)
HALF_A = CORES * NPCA      # 28672 rows in table A (< 32768 for int16)
HALF_B = CORES * NPCB      # 21504 rows in table B
KT = 30 if DR else 29      # k-chunks of 128 (30 = even count for DoubleRow)
KPAD = KT * P              # 3840 / 3712
NB = 512                   # phase-1 node block
S2W = 128                  # s2 table row: 128 bf16 = 256B (first 6 used)
GB1 = 8                    # chunks per batched dma_gather call (L1, 512B rows)
GB2 = 8                    # chunks per batched dma_gather call (L2, 256B rows)

LAST_RESULT = None         # BassKernelResults of the most recent run
_CACHE = {}                # (CL, CU) -> compiled Bacc


def _idx_cols(CL, CU):
    return TILES * (CL + CU) * 8


def _build(CL, CU, num_devices=CORES, with_ag=True,
           phases=("p1", "p3", "p5", "p6")):
    CH = CL + CU
    idx_cols = _idx_cols(CL, CU)

    nc = bacc.Bacc("TRN2", target_bir_lowering=False, debug=False,
                   num_devices=num_devices)

    XDT = FP8 if X_FP8 else BF16
    W1DT = FP8 if DR else BF16
    xT = nc.dram_tensor("xT", [KPAD, NPC], XDT, kind="ExternalInput")
    W1p = nc.dram_tensor("W1p", [KPAD, F_HID], W1DT, kind="ExternalInput")
    W2p = nc.dram_tensor("W2p", [F_HID, F_OUT], BF16, kind="ExternalInput")
    b1t = nc.dram_tensor("b1t", [P, 2], F32, kind="ExternalInput")
    b2b = nc.dram_tensor("b2b", [P, F_OUT], F32, kind="ExternalInput")
    idxd = nc.dram_tensor("idxd", [P, idx_cols], I16, kind="ExternalInput")
    idx2d = nc.dram_tensor("idx2d", [P, idx_cols], I16, kind="ExternalInput")
    dstd = nc.dram_tensor("dstd", [P, TILES * CH], F32, kind="ExternalInput")
    wd = nc.dram_tensor("wd", [P, TILES * CH], F32, kind="ExternalInput")
    outd = nc.dram_tensor("out", [NPC, F_OUT], F32, kind="ExternalOutput")

    ag1_in = nc.dram_tensor("ag1_in", [NPC, F_HID], BF16, kind="Internal")
    ag1_outA = nc.dram_tensor("ag1_outA", [HALF_A, F_HID], BF16,
                              kind="Internal", addr_space="Shared")
    ag1_outB = nc.dram_tensor("ag1_outB", [HALF_B, F_HID], BF16,
                              kind="Internal", addr_space="Shared")
    ag2_inA = nc.dram_tensor("ag2_inA", [NPCA, S2W], BF16, kind="Internal")
    ag2_inB = nc.dram_tensor("ag2_inB", [NPCB, S2W], BF16, kind="Internal")
    ag2_outA = nc.dram_tensor("ag2_outA", [HALF_A, S2W], BF16,
                              kind="Internal", addr_space="Shared")
    ag2_outB = nc.dram_tensor("ag2_outB", [HALF_B, S2W], BF16,
                              kind="Internal", addr_space="Shared")

    rg = [list(range(num_devices))]

    def ag(in_ap, out_ap):
        if with_ag:
            nc.gpsimd.collective_compute(
                "AllGather", mybir.AluOpType.bypass, replica_groups=rg,
                ins=[in_ap], outs=[out_ap])
        else:
            nc.sync.dma_start(out=out_ap[0:in_ap.shape[0], :], in_=in_ap)

    with tile.TileContext(nc) as tc:
        with (
            tc.tile_pool(name="res", bufs=1) as rp,
            tc.tile_pool(name="mt", bufs=32) as mp,
        ):
            # ---------- resident constants ----------
            iota_i = rp.tile([P, P], mybir.dt.int32)
            nc.gpsimd.iota(iota_i[:], pattern=[[1, P]], base=0,
                           channel_multiplier=0)
            iota_bf = rp.tile([P, P], BF16)
            nc.vector.tensor_copy(iota_bf[:], iota_i[:])
            # preload all three ACT function tables up front so no
            # LoadActFuncSet lands on the critical path later
            scr = rp.tile([P, 1], F32)
            for fn in (mybir.ActivationFunctionType.Lrelu if LRELU
                       else mybir.ActivationFunctionType.Relu,
                       mybir.ActivationFunctionType.Exp,
                       mybir.ActivationFunctionType.Ln):
                nc.scalar.activation(scr[:], iota_bf[:, 0:1], fn, alpha=0.01)

            w2sb = rp.tile([P, 2, F_OUT], BF16)
            nc.sync.dma_start(
                out=w2sb[:], in_=W2p[:, :].rearrange("(k p) n -> p k n", p=P))
            b1sb = rp.tile([P, 2], F32)
            nc.sync.dma_start(out=b1sb[:], in_=b1t[:, :])
            b2sb = rp.tile([P, F_OUT], F32)
            nc.sync.dma_start(out=b2sb[:], in_=b2b[:, :])
            idxsb = rp.tile([P, idx_cols], I16)
            nc.sync.dma_start(out=idxsb[:], in_=idxd[:, :])
            idx2sb = rp.tile([P, idx_cols], I16)
            nc.sync.dma_start(out=idx2sb[:], in_=idx2d[:, :])
            dstsb = rp.tile([P, TILES * CH], F32)
            nc.sync.dma_start(out=dstsb[:], in_=dstd[:, :])
            wsb = rp.tile([P, TILES * CH], F32)
            nc.sync.dma_start(out=wsb[:], in_=wd[:, :])

            NCHL = TILES * CL
            NCHU = TILES * CU

            class Stream:
                """Batched dma_gather stream over one table half."""

                def __init__(self, pool, tag, idx_sb, tab, nch, base_col,
                             esz, gbsz):
                    self.pool, self.tag = pool, tag
                    self.idx_sb, self.tab = idx_sb, tab
                    self.nch, self.base_col = nch, base_col
                    self.esz, self.gbsz = esz, gbsz
                    self.next = 0
                    self.bufs = {}

                def issue(self):
                    k = self.next
                    c0 = k * self.gbsz
                    n = min(self.gbsz, self.nch - c0)
                    gb = self.pool.tile([P, n, self.esz], BF16, tag=self.tag)
                    nc.gpsimd.dma_gather(
                        out_ap=gb[:], in_ap=self.tab,
                        idxs_ap=self.idx_sb[:, (self.base_col + c0) * 8:
                                            (self.base_col + c0 + n) * 8],
                        num_idxs=n * P, num_idxs_reg=n * P,
                        elem_size=self.esz)
                    self.bufs[k] = gb
                    self.next += 1

                def get(self, g):
                    while self.next * self.gbsz <= g:
                        self.issue()
                    return self.bufs[g // self.gbsz], g % self.gbsz

            # L1-A gather pool is allocated BEFORE phase 1's pools so the
            # early table-A gathers can run during phase 1's PE-bound tail
            # (no false SBUF-reuse dependency on the x strips).
            with tc.tile_pool(name="gbL", bufs=14) as gLp:

                # ---------- phase 1: support = x @ W1 ----------
                if "p1" in phases:
                    with (
                        tc.tile_pool(name="p1w", bufs=1) as p1w,
                        tc.tile_pool(name="p1x", bufs=3) as p1x,
                        tc.tile_pool(name="p1ps", bufs=4, space="PSUM")
                        as p1ps,
                    ):
                        XDT_SB = FP8 if X_FP8 else BF16
                        w1sb = p1w.tile([P, KT, F_HID], W1DT)
                        nc.sync.dma_start(
                            out=w1sb[:],
                            in_=W1p[:, :].rearrange("(k p) n -> p k n", p=P))
                        blocks = []
                        b0 = 0
                        while b0 < NPC:
                            bsz = min(NB,
                                      (NPCA - b0) if b0 < NPCA else (NPC - b0))
                            blocks.append((b0, bsz))
                            b0 += bsz
                        for b0, bsz in blocks:
                            xsb = p1x.tile([P, KT, bsz], XDT_SB, tag="xsb")
                            nc.sync.dma_start(
                                out=xsb[:],
                                in_=xT[:, b0:b0 + bsz].rearrange(
                                    "(k p) n -> p k n", p=P))
                            nm = bsz // P
                            sup = p1x.tile([P, nm, F_HID], BF16, tag="sup")
                            for m in range(nm):
                                ps = p1ps.tile([P, F_HID], F32, tag="p1")
                                if DR:
                                    for k in range(KT // 2):
                                        nc.tensor.matmul(
                                            ps[:],
                                            lhsT=xsb[:, 2 * k:2 * k + 2,
                                                     m * P:(m + 1) * P],
                                            rhs=w1sb[:, 2 * k:2 * k + 2, :],
                                            start=(k == 0),
                                            stop=(k == KT // 2 - 1),
                                            perf_mode=mybir.MatmulPerfMode
                                            .DoubleRow)
                                else:
                                    for k in range(KT):
                                        nc.tensor.matmul(
                                            ps[:],
                                            lhsT=xsb[:, k, m * P:(m + 1) * P],
                                            rhs=w1sb[:, k, :],
                                            start=(k == 0),
                                            stop=(k == KT - 1))
                                nc.vector.tensor_copy(sup[:, m, :], ps[:])
                            nc.sync.dma_start(
                                out=ag1_in[b0:b0 + bsz, :].rearrange(
                                    "(m p) f -> p m f", p=P),
                                in_=sup[:])
                            if b0 + bsz == NPCA:
                                ag(ag1_in[0:NPCA, :], ag1_outA[:, :])
                        ag(ag1_in[NPCA:NPC, :], ag1_outB[:, :])
                elif "p3" in phases:
                    ag(ag1_in[0:NPCA, :], ag1_outA[:, :])
                    ag(ag1_in[NPCA:NPC, :], ag1_outB[:, :])

                with (
                    tc.tile_pool(name="big", bufs=1) as bigp,
                    tc.tile_pool(name="work", bufs=8) as wp,
                    tc.tile_pool(name="g2L", bufs=14) as g2Lp,
                    tc.tile_pool(name="g2U", bufs=8) as g2Up,
                    tc.tile_pool(name="ps", bufs=2, space="PSUM") as pp,
                ):
                    h1T = bigp.tile([P, 2, TILES * P], BF16)
                    s2rows = bigp.tile([P, TILES, 8], BF16)
                    if "p5" in phases:
                        nc.gpsimd.memset(s2rows[:], 0.0)
                    if "p3" not in phases and "p5" in phases:
                        nc.gpsimd.memset(h1T[:], 0.0)

                    g1L = Stream(gLp, "g1L", idxsb, ag1_outA[:, :], NCHL,
                                 0, F_HID, GB1)
                    g1U = Stream(wp, "g1U", idxsb, ag1_outB[:, :], NCHU,
                                 NCHL, F_HID, GB1)
                    g2L = Stream(g2Lp, "g2L", idx2sb, ag2_outA[:, :], NCHL,
                                 0, S2W, GB2)
                    g2U = Stream(g2Up, "g2U", idx2sb, ag2_outB[:, :], NCHU,
                                 NCHL, S2W, GB2)

                    def mk_get(sL, sU):
                        def get(t, c):
                            if c < CL:
                                return sL.get(t * CL + c)
                            return sU.get(t * CU + (c - CL))
                        return get

                    def mt_build(tag, col):
                        mt = mp.tile([P, P], BF16, tag=tag)
                        nc.vector.tensor_scalar(
                            out=mt[:], in0=iota_bf[:],
                            scalar1=dstsb[:, col:col + 1],
                            scalar2=wsb[:, col:col + 1],
                            op0=mybir.AluOpType.is_equal,
                            op1=mybir.AluOpType.mult)
                        return mt

                    def s2_tile(t):
                        ps5 = pp.tile([P, F_OUT], F32, tag="ps5")
                        for h in range(2):
                            nc.tensor.matmul(
                                ps5[:], lhsT=h1T[:, h, t * P:(t + 1) * P],
                                rhs=w2sb[:, h, :], start=(h == 0),
                                stop=(h == 1))
                        nc.vector.tensor_copy(s2rows[:, t, 0:F_OUT], ps5[:])
                        if t == TILES_A - 1:
                            nc.sync.dma_start(
                                out=ag2_inA[:, 0:8].rearrange(
                                    "(p t) f -> p t f", p=P),
                                in_=s2rows[:, 0:TILES_A, :])
                            ag(ag2_inA[:, :], ag2_outA[:, :])
                        elif TILES - 5 > TILES_A and t == TILES - 5:
                            nc.sync.dma_start(
                                out=ag2_inB[:, 0:8].rearrange(
                                    "(p t) f -> p t f",
                                    p=P)[:, 0:TILES_B - 4, :],
                                in_=s2rows[:, TILES_A:TILES - 4, :])
                        elif t == TILES - 1:
                            bsplit = (TILES - 4 if TILES - 5 > TILES_A
                                      else TILES_A)
                            nc.sync.dma_start(
                                out=ag2_inB[:, 0:8].rearrange(
                                    "(p t) f -> p t f",
                                    p=P)[:, bsplit - TILES_A:TILES_B, :],
                                in_=s2rows[:, bsplit:TILES, :])
                            ag(ag2_inB[:, :], ag2_outB[:, :])

                    # ------ phase 3: L1 aggregation (+ fused phase 5) ------
                    def l1_tile(t, get):
                        psT0 = pp.tile([P, P], F32, tag="psT0")
                        psT1 = pp.tile([P, P], F32, tag="psT1")
                        psT = [psT0, psT1]
                        for c in range(CH):
                            gb, slot = get(t, c)
                            mt = mt_build("mt1", t * CH + c)
                            for h in range(2):
                                nc.tensor.matmul(
                                    psT[h][:],
                                    lhsT=gb[:, slot, h * P:(h + 1) * P],
                                    rhs=mt[:],
                                    start=(c == 0), stop=(c == CH - 1))
                        for h in range(2):
                            nc.scalar.activation(
                                h1T[:, h, t * P:(t + 1) * P], psT[h][:],
                                mybir.ActivationFunctionType.Lrelu if LRELU
                                else mybir.ActivationFunctionType.Relu,
                                bias=b1sb[:, h:h + 1],
                                scale=(1.0 / W1_SCALE) if DR else 1.0,
                                alpha=0.01)
                        if "p5" in phases:
                            s2_tile(t)

                    if "p3" in phases:
                        get1 = mk_get(g1L, g1U)
                        for _ in range(12):  # table-A gathers run in p1 tail
                            if g1L.next * g1L.gbsz < g1L.nch:
                                g1L.issue()
                        for t in range(TILES):
                            l1_tile(t, get1)
                            if "p5" in phases and "p6" in phases \
                                    and t >= TILES_A + 4 and t % 2 == 0 \
                                    and g2L.next < 14 \
                                    and g2L.next * g2L.gbsz < g2L.nch:
                                g2L.issue()  # L2-A gathers run in p3 tail
                    elif "p5" in phases:
                        for t in range(TILES):
                            s2_tile(t)

                    # ---------- phase 6: L2 aggregation ----------
                    zall = bigp.tile([P, TILES * F_OUT], F32)

                    def l2_tile(t, get):
                        ps = pp.tile([P, F_OUT], F32, tag="ps6")
                        for c in range(CH):
                            gb, slot = get(t, c)
                            mt = mt_build("mt2", t * CH + c)
                            nc.tensor.matmul(
                                ps[:], lhsT=mt[:], rhs=gb[:, slot, 0:F_OUT],
                                start=(c == 0), stop=(c == CH - 1))
                        nc.vector.tensor_tensor(
                            out=zall[:, t * F_OUT:(t + 1) * F_OUT],
                            in0=ps[:], in1=b2sb[:], op=mybir.AluOpType.add)

                    # ---- phase 7: log_softmax + output, in groups so the
                    # tail overlaps the last L2 gathers ----
                    def softmax_group(lo, hi):
                        n = hi - lo
                        zv = zall[:, lo * F_OUT:hi * F_OUT].rearrange(
                            "p (t f) -> p t f", f=F_OUT)
                        mx = wp.tile([P, n], F32, tag="mx")
                        nc.vector.tensor_reduce(out=mx[:], in_=zv,
                                                op=mybir.AluOpType.max,
                                                axis=mybir.AxisListType.X)
                        tsub = wp.tile([P, n, F_OUT], F32, tag="tsub")
                        nc.vector.tensor_tensor(
                            out=tsub[:], in0=zv,
                            in1=mx[:][:, :, None].to_broadcast([P, n, F_OUT]),
                            op=mybir.AluOpType.subtract)
                        ex = wp.tile([P, n, F_OUT], F32, tag="ex")
                        nc.scalar.activation(ex[:], tsub[:],
                                             mybir.ActivationFunctionType.Exp)
                        sm = wp.tile([P, n], F32, tag="sm")
                        nc.vector.tensor_reduce(out=sm[:], in_=ex[:],
                                                op=mybir.AluOpType.add,
                                                axis=mybir.AxisListType.X)
                        ls = wp.tile([P, n], F32, tag="ls")
                        nc.scalar.activation(ls[:], sm[:],
                                             mybir.ActivationFunctionType.Ln)
                        res = wp.tile([P, n, F_OUT], F32, tag="res")
                        nc.vector.tensor_tensor(
                            out=res[:], in0=tsub[:],
                            in1=ls[:][:, :, None].to_broadcast([P, n, F_OUT]),
                            op=mybir.AluOpType.subtract)
                        # p-major output rows: one contiguous (hi-lo)*24B
                        # descriptor per partition instead of 24B descriptors
                        nc.sync.dma_start(
                            out=outd[:, :].rearrange(
                                "(p t) f -> p t f", t=TILES)[:, lo:hi, :],
                            in_=res[:])

                    GRP = sorted(set([0, (TILES * 22) // 49,
                                      (TILES * 44) // 49, TILES]))
                    if "p6" in phases:
                        if "p5" not in phases:
                            ag(ag2_inA[:, :], ag2_outA[:, :])
                            ag(ag2_inB[:, :], ag2_outB[:, :])
                        get2 = mk_get(g2L, g2U)
                        gi = 1
                        for t in range(TILES):
                            l2_tile(t, get2)
                            if t + 1 == GRP[gi]:
                                softmax_group(GRP[gi - 1], GRP[gi])
                                gi += 1
                    else:
                        nc.gpsimd.memset(zall[:], 0.0)
                        for gi in range(1, len(GRP)):
                            softmax_group(GRP[gi - 1], GRP[gi])

    nc.compile()
    return nc


def _preprocess(x, edge_src, edge_dst, edge_weight, W1, b1, W2, b2):
    import ml_dtypes
    bf16 = ml_dtypes.bfloat16
    xdt = ml_dtypes.float8_e4m3 if X_FP8 else bf16

    x = np.asarray(x, dtype=np.float32)
    edge_src = np.asarray(edge_src, dtype=np.int64)
    edge_dst = np.asarray(edge_dst, dtype=np.int64)
    edge_weight = np.asarray(edge_weight, dtype=np.float32)
    W1 = np.asarray(W1, dtype=np.float32)
    b1 = np.asarray(b1, dtype=np.float32)
    W2 = np.asarray(W2, dtype=np.float32)
    b2 = np.asarray(b2, dtype=np.float32)

    NBINS = CORES * TILES
    deg = np.bincount(edge_dst, minlength=N_NODES)

    # degree-balanced assignment of nodes to bins (LPT with 128-node cap)
    import heapq
    order = np.argsort(-deg, kind="stable")
    heap = [(0, b) for b in range(NBINS)]
    heapq.heapify(heap)
    counts = np.zeros(NBINS, dtype=np.int64)
    node_row = np.empty(N_NODES, dtype=np.int64)
    for nid in order:
        while True:
            load, b = heapq.heappop(heap)
            if counts[b] < P:
                break
        core, t = b // TILES, b % TILES
        node_row[nid] = core * NPC + t * P + counts[b]
        counts[b] += 1
        if counts[b] < P:
            heapq.heappush(heap, (load + int(deg[nid]), b))

    src_row = node_row[edge_src]
    dst_row = node_row[edge_dst]
    core_e = dst_row // NPC
    t_e = (dst_row % NPC) // P
    lane_d = dst_row % P
    src_core = src_row // NPC
    src_local = src_row % NPC
    half_e = (src_local >= NPCA).astype(np.int64)
    # L1 table rows: node order within each half
    loc_src = np.where(half_e == 0, src_core * NPCA + src_local,
                       src_core * NPCB + (src_local - NPCA))
    # L2 table rows: p-major within each half ((p, t) -> p*TILES_half + t)
    t_s = src_local // P
    p_s = src_local % P
    loc2_src = np.where(
        half_e == 0,
        src_core * NPCA + p_s * TILES_A + t_s,
        src_core * NPCB + p_s * TILES_B + (t_s - TILES_A))

    # position of each edge within its (core,tile,half) run
    key = (core_e * TILES + t_e) * 2 + half_e
    sort_i = np.argsort(key, kind="stable")
    ks = key[sort_i]
    cnt = np.bincount(ks, minlength=NBINS * 2)
    starts = np.zeros(NBINS * 2, dtype=np.int64)
    starts[1:] = np.cumsum(cnt)[:-1]
    pos_sorted = np.arange(N_EDGES) - starts[ks]
    pos = np.empty(N_EDGES, dtype=np.int64)
    pos[sort_i] = pos_sorted

    nL = cnt[0::2].reshape(CORES, TILES)
    nU = cnt[1::2].reshape(CORES, TILES)
    CL = max(1, int(np.ceil(nL.max() / P)))
    CU = max(1, int(np.ceil(nU.max() / P)))
    CH = CL + CU
    idx_cols = _idx_cols(CL, CU)

    g_stream = np.where(half_e == 0, t_e * CL + pos // P,
                        t_e * CU + pos // P)
    sbase = np.where(half_e == 0, 0, TILES * CL * 8)
    lane_s = pos % P
    idx_col = sbase + g_stream * 8 + lane_s // 16
    idx_par = lane_s % 16

    idx_arr = np.zeros((CORES, 16, idx_cols), dtype=np.int16)
    idx_arr[core_e, idx_par, idx_col] = loc_src.astype(np.int16)
    idx2_arr = np.zeros((CORES, 16, idx_cols), dtype=np.int16)
    idx2_arr[core_e, idx_par, idx_col] = loc2_src.astype(np.int16)

    # per-(tile,chunk) Mt data
    # p-major output row for node_row (kernel writes out[(p t), f])
    r_n = node_row % NPC
    out_row = (node_row // NPC) * NPC + (r_n % P) * TILES + (r_n // P)

    c_e = np.where(half_e == 0, pos // P, CL + pos // P)
    lane_e = pos % P
    dcol = t_e * CH + c_e
    dst_arr = np.zeros((CORES, P, TILES * CH), dtype=np.float32)
    w_arr = np.zeros((CORES, P, TILES * CH), dtype=np.float32)
    dst_arr[core_e, lane_e, dcol] = lane_d.astype(np.float32)
    w_arr[core_e, lane_e, dcol] = edge_weight

    W1p = np.zeros((KPAD, F_HID), dtype=np.float32)
    W1p[:F_IN] = W1
    if DR:
        w1dt = ml_dtypes.float8_e4m3
        W1p_c = np.ascontiguousarray((W1p * W1_SCALE).astype(w1dt))
    else:
        W1p_c = np.ascontiguousarray(W1p.astype(bf16))
    b1t = np.ascontiguousarray(b1.reshape(2, P).T.astype(np.float32))
    b2b = np.broadcast_to(b2, (P, F_OUT)).copy()

    in_maps = []
    row_node = np.full(NTOT, -1, dtype=np.int64)
    row_node[node_row] = np.arange(N_NODES)
    for c in range(CORES):
        rows = row_node[c * NPC:(c + 1) * NPC]
        xc = np.zeros((NPC, F_IN), dtype=np.float32)
        occ = rows >= 0
        xc[occ] = x[rows[occ]]
        xTc = np.zeros((KPAD, NPC), dtype=xdt)
        xTc[:F_IN] = xc.T.astype(xdt)
        in_maps.append(dict(
            xT=xTc,
            W1p=W1p_c,
            W2p=np.ascontiguousarray(W2.astype(bf16)),
            b1t=b1t,
            b2b=b2b,
            idxd=np.tile(idx_arr[c], (8, 1)),
            idx2d=np.tile(idx2_arr[c], (8, 1)),
            dstd=dst_arr[c],
            wd=w_arr[c],
        ))
    return in_maps, out_row, CL, CU


def kernel(**inputs):
    global LAST_RESULT
    in_maps, node_row, CL, CU = _preprocess(**inputs)
    key = (CL, CU)
    if key not in _CACHE:
        _CACHE[key] = _build(CL, CU)
    nc = _CACHE[key]
    res = run_bass_kernel_spmd(nc, in_maps, core_ids=list(range(CORES)))
    LAST_RESULT = res
    allout = np.concatenate([res.results[c]["out"] for c in range(CORES)],
                            axis=0)
    return np.ascontiguousarray(allout[node_row]).astype(np.float32)
